# revision 11
# baseline (speedup 1.0000x reference)
"""CardHandEmbed kernel for 8 Trainium2 NeuronCores.

Strategy (moe_routing): the reference computes all 9 type-heads for every
token and one-hot selects one.  card_type < 8, so only heads 0..7 are ever
selected.  We expert-shard: core t processes exactly the tokens with
card_type == t (routing/permutation done on host), so each core needs only
its own head weights and computes 1/9th of the reference head FLOPs.

Per-core device pipeline (tokens live on the free axis, features on
partitions, i.e. everything is computed transposed):
  - id embedding: indirect-DMA row gather from the (replicated) 100k-row
    table, then PE transposes into feature-major layout.
  - cost embedding: folded into the head matmul.  cost_vec @ Wh_cost ==
    onehot(cost) @ (blockdiag(cost_table)^T @ Wh_cost); the host packs the
    one-hot rows and folds the table into the weight.
  - type embedding: constant per core -> folded into the head bias.
  - cont MLP: the 9 engineered features are affine in
    [cost6, atk, hp, ha, hh] except r1=atk*hp, r2=atk/(mana+eps),
    r3=hp/(mana+eps); the affine part is folded into Wc1 on the host and
    r1..r3 are computed on device and added via rank-1 accumulating matmuls.
  - head matmul (K=480 after folding), tanh, fuse matmul (K=768), tanh.
Host then inverse-permutes the per-core outputs into [B, N, E].
"""

import os

import numpy as np

import concourse.bacc as bacc
import concourse.bass as bass
import concourse.tile as tile
from concourse import mybir

B, N = 64, 128
T = B * N
MAX_ID = 100000
ID_DIM = 256
MAX_TYPE = 8
N_HEADS = MAX_TYPE + 1
TYPE_DIM = 64
COST_LEN = 16
MANA_DIM = 32
N_COST = 6
SP_LEN = 32
CAT_DIM = 128
CONT_HID = 64
EMBED_DIM = 512
HEAD_IN = ID_DIM + TYPE_DIM + MANA_DIM * N_COST + CAT_DIM  # 640
EPS = 1e-6
N_CORES = 8
F32 = mybir.dt.float32
I32 = mybir.dt.int32

# filled by kernel() for test harness inspection
LAST_RESULTS = None


def _chunks(C, step=512):
    out = []
    c0 = 0
    while c0 < C:
        out.append((c0, min(step, C - c0)))
        c0 += step
    return out


def _build(C):
    """Build the (SPMD-uniform) Bass program for per-core capacity C."""
    nG = C // 128
    nc = bacc.Bacc("TRN2")

    # ---- DRAM I/O ----
    d_table = nc.dram_tensor("id_table", [MAX_ID, ID_DIM], F32, kind="ExternalInput")
    d_ids = nc.dram_tensor("ids", [128, nG], I32, kind="ExternalInput")
    d_ftsp = nc.dram_tensor("ftsp", [32, C], F32, kind="ExternalInput")
    d_ftc = nc.dram_tensor("ftc", [11, C], F32, kind="ExternalInput")
    d_oh = nc.dram_tensor("oh", [96, C], F32, kind="ExternalInput")
    d_atk = nc.dram_tensor("atk2", [1, C], F32, kind="ExternalInput")
    d_hp = nc.dram_tensor("hp2", [1, C], F32, kind="ExternalInput")
    d_wh = nc.dram_tensor("wh", [480, EMBED_DIM], F32, kind="ExternalInput")
    d_bh = nc.dram_tensor("bh4", [128, 4], F32, kind="ExternalInput")
    d_wsp = nc.dram_tensor("wsp", [SP_LEN, CAT_DIM], F32, kind="ExternalInput")
    d_bsp = nc.dram_tensor("bsp", [128, 1], F32, kind="ExternalInput")
    d_wc1a = nc.dram_tensor("wc1a", [11, CONT_HID], F32, kind="ExternalInput")
    d_wc1r1 = nc.dram_tensor("wc1r1", [1, CONT_HID], F32, kind="ExternalInput")
    d_wc1r2 = nc.dram_tensor("wc1r2", [1, CONT_HID], F32, kind="ExternalInput")
    d_wc1r3 = nc.dram_tensor("wc1r3", [1, CONT_HID], F32, kind="ExternalInput")
    d_bc1 = nc.dram_tensor("bc1", [CONT_HID, 1], F32, kind="ExternalInput")
    d_wc2 = nc.dram_tensor("wc2", [CONT_HID, ID_DIM], F32, kind="ExternalInput")
    d_bc2 = nc.dram_tensor("bc2p", [128, 2], F32, kind="ExternalInput")
    d_wf = nc.dram_tensor("wf", [EMBED_DIM + ID_DIM, EMBED_DIM], F32, kind="ExternalInput")
    d_bf = nc.dram_tensor("bfp", [128, 4], F32, kind="ExternalInput")
    d_out = nc.dram_tensor("out", [EMBED_DIM, C], F32, kind="ExternalOutput")

    ident_h = nc.inline_tensor(np.eye(128, dtype=np.float32), name="ident")
    mlhs_np = np.zeros((11, 1), dtype=np.float32)
    mlhs_np[0:6, 0] = 1.0
    mlhs_np[10, 0] = EPS
    mlhs_h = nc.inline_tensor(mlhs_np, name="mlhs")

    Tanh = mybir.ActivationFunctionType.Tanh

    with tile.TileContext(nc) as tc:
        from contextlib import ExitStack

        with ExitStack() as ctx:
            pers = ctx.enter_context(tc.tile_pool(name="pers", bufs=1))
            gat = ctx.enter_context(tc.tile_pool(name="gat", bufs=4))
            ppt = ctx.enter_context(tc.tile_pool(name="ppt", bufs=2, space="PSUM"))
            pmm = ctx.enter_context(tc.tile_pool(name="pmm", bufs=4, space="PSUM"))
            pmana = ctx.enter_context(tc.tile_pool(name="pmana", bufs=1, space="PSUM"))

            # ---- persistent SBUF tiles + loads ----
            ids_sb = pers.tile([128, nG], I32, name="ids_sb")
            nc.sync.dma_start(out=ids_sb[:], in_=d_ids[:])

            ident_sb = pers.tile([128, 128], F32, name="ident_sb")
            nc.sync.dma_start(out=ident_sb[:], in_=ident_h[:])
            mlhs_sb = pers.tile([11, 1], F32, name="mlhs_sb")
            nc.sync.dma_start(out=mlhs_sb[:], in_=mlhs_h[:])

            wh_sb = []
            bounds = [(0, 128), (128, 256), (256, 384), (384, 480)]
            for k, (r0, r1) in enumerate(bounds):
                w = pers.tile([r1 - r0, EMBED_DIM], F32, name=f"wh{k}_sb")
                nc.sync.dma_start(out=w[:], in_=d_wh[r0:r1, :])
                wh_sb.append(w)
            wf_sb = []
            for k in range(6):
                w = pers.tile([128, EMBED_DIM], F32, name=f"wf{k}_sb")
                nc.sync.dma_start(out=w[:], in_=d_wf[k * 128 : (k + 1) * 128, :])
                wf_sb.append(w)
            wsp_sb = pers.tile([SP_LEN, CAT_DIM], F32, name="wsp_sb")
            nc.sync.dma_start(out=wsp_sb[:], in_=d_wsp[:])
            wc1a_sb = pers.tile([11, CONT_HID], F32, name="wc1a_sb")
            nc.sync.dma_start(out=wc1a_sb[:], in_=d_wc1a[:])
            wc1r1_sb = pers.tile([1, CONT_HID], F32, name="wc1r1_sb")
            nc.sync.dma_start(out=wc1r1_sb[:], in_=d_wc1r1[:])
            wc1r2_sb = pers.tile([1, CONT_HID], F32, name="wc1r2_sb")
            nc.sync.dma_start(out=wc1r2_sb[:], in_=d_wc1r2[:])
            wc1r3_sb = pers.tile([1, CONT_HID], F32, name="wc1r3_sb")
            nc.sync.dma_start(out=wc1r3_sb[:], in_=d_wc1r3[:])
            wc2_sb = pers.tile([CONT_HID, ID_DIM], F32, name="wc2_sb")
            nc.sync.dma_start(out=wc2_sb[:], in_=d_wc2[:])
            bh_sb = pers.tile([128, 4], F32, name="bh_sb")
            nc.sync.dma_start(out=bh_sb[:], in_=d_bh[:])
            bsp_sb = pers.tile([128, 1], F32, name="bsp_sb")
            nc.sync.dma_start(out=bsp_sb[:], in_=d_bsp[:])
            bc1_sb = pers.tile([CONT_HID, 1], F32, name="bc1_sb")
            nc.sync.dma_start(out=bc1_sb[:], in_=d_bc1[:])
            bc2_sb = pers.tile([128, 2], F32, name="bc2_sb")
            nc.sync.dma_start(out=bc2_sb[:], in_=d_bc2[:])
            bf_sb = pers.tile([128, 4], F32, name="bf_sb")
            nc.sync.dma_start(out=bf_sb[:], in_=d_bf[:])

            ftsp_sb = pers.tile([32, C], F32, name="ftsp_sb")
            nc.sync.dma_start(out=ftsp_sb[:], in_=d_ftsp[:])
            ftc_sb = pers.tile([11, C], F32, name="ftc_sb")
            nc.sync.dma_start(out=ftc_sb[:], in_=d_ftc[:])
            atk_sb = pers.tile([1, C], F32, name="atk_sb")
            nc.sync.dma_start(out=atk_sb[:], in_=d_atk[:])
            hp_sb = pers.tile([1, C], F32, name="hp_sb")
            nc.sync.dma_start(out=hp_sb[:], in_=d_hp[:])

            # head-input feature-major tiles (the 4 K-chunks of head matmul)
            X0 = pers.tile([128, C], F32, name="X0")
            X1 = pers.tile([128, C], F32, name="X1")
            X2 = pers.tile([128, C], F32, name="X2")
            X3 = pers.tile([96, C], F32, name="X3")
            nc.sync.dma_start(out=X2[0:96, :], in_=d_oh[:])

            CH = pers.tile([CONT_HID, C], F32, name="CH")
            CV0 = pers.tile([128, C], F32, name="CV0")
            CV1 = pers.tile([128, C], F32, name="CV1")
            RECIP = pers.tile([1, C], F32, name="RECIP")
            R1 = pers.tile([1, C], F32, name="R1")
            R2 = pers.tile([1, C], F32, name="R2")
            R3 = pers.tile([1, C], F32, name="R3")
            TO = [pers.tile([128, C], F32, name=f"TO{m}") for m in range(4)]
            OUT = [pers.tile([128, C], F32, name=f"OUT{m}") for m in range(4)]

            # ---- id gather + transpose into X0/X1 ----
            for g in range(nG):
                rows = gat.tile([128, ID_DIM], F32, name="grows", tag="grows")
                nc.gpsimd.indirect_dma_start(
                    out=rows[:],
                    out_offset=None,
                    in_=d_table[:],
                    in_offset=bass.IndirectOffsetOnAxis(ap=ids_sb[:, g : g + 1], axis=0),
                )
                for h, Xh in enumerate((X0, X1)):
                    pt = ppt.tile([128, 128], F32, name="pt", tag="pt")
                    nc.tensor.transpose(
                        out=pt[:], in_=rows[:, h * 128 : (h + 1) * 128], identity=ident_sb[:]
                    )
                    nc.vector.tensor_copy(
                        out=Xh[:, g * 128 : (g + 1) * 128], in_=pt[:]
                    )

            # ---- per 512-token chunk pipeline ----
            for c0, cw in _chunks(C):
                sl = slice(c0, c0 + cw)
                # special_vec = tanh(Xsp @ Wsp + bsp), feature-major
                ps = pmm.tile([128, 512], F32, name="ps", tag="mm")
                nc.tensor.matmul(
                    out=ps[:, :cw], lhsT=wsp_sb[:], rhs=ftsp_sb[:, sl],
                    start=True, stop=True,
                )
                nc.scalar.activation(
                    out=X2[96:128, sl], in_=ps[96:128, :cw], func=Tanh,
                    bias=bsp_sb[96:128, :],
                )
                nc.scalar.activation(
                    out=X3[0:96, sl], in_=ps[0:96, :cw], func=Tanh,
                    bias=bsp_sb[0:96, :],
                )

                # mana + eps, reciprocal, nonlinear cont features
                pm = pmana.tile([1, 512], F32, name="pm", tag="pm")
                nc.tensor.matmul(
                    out=pm[:, :cw], lhsT=mlhs_sb[:], rhs=ftc_sb[:, sl],
                    start=True, stop=True,
                )
                nc.vector.reciprocal(out=RECIP[:, sl], in_=pm[:, :cw])
                nc.vector.tensor_mul(R1[:, sl], atk_sb[:, sl], hp_sb[:, sl])
                nc.vector.tensor_mul(R2[:, sl], atk_sb[:, sl], RECIP[:, sl])
                nc.vector.tensor_mul(R3[:, sl], hp_sb[:, sl], RECIP[:, sl])

                # cont layer 1: affine part + three rank-1 nonlinear rows
                pc1 = pmm.tile([128, 512], F32, name="pc1", tag="mm")
                nc.tensor.matmul(
                    out=pc1[0:CONT_HID, :cw], lhsT=wc1a_sb[:], rhs=ftc_sb[:, sl],
                    start=True, stop=False,
                )
                nc.tensor.matmul(
                    out=pc1[0:CONT_HID, :cw], lhsT=wc1r1_sb[:], rhs=R1[:, sl],
                    start=False, stop=False,
                )
                nc.tensor.matmul(
                    out=pc1[0:CONT_HID, :cw], lhsT=wc1r2_sb[:], rhs=R2[:, sl],
                    start=False, stop=False,
                )
                nc.tensor.matmul(
                    out=pc1[0:CONT_HID, :cw], lhsT=wc1r3_sb[:], rhs=R3[:, sl],
                    start=False, stop=True,
                )
                nc.scalar.activation(
                    out=CH[:, sl], in_=pc1[0:CONT_HID, :cw], func=Tanh, bias=bc1_sb[:],
                )

                # cont layer 2 -> cont_vec (256 dims = CV0, CV1)
                for m, CVm in enumerate((CV0, CV1)):
                    pc2 = pmm.tile([128, 512], F32, name="pc2", tag="mm")
                    nc.tensor.matmul(
                        out=pc2[:, :cw], lhsT=wc2_sb[:, m * 128 : (m + 1) * 128],
                        rhs=CH[:, sl], start=True, stop=True,
                    )
                    nc.scalar.activation(
                        out=CVm[:, sl], in_=pc2[:, :cw], func=Tanh,
                        bias=bc2_sb[:, m : m + 1],
                    )

                # routed head matmul: K = 480 over 4 chunks
                Xs = (X0, X1, X2, X3)
                for m in range(4):
                    ph = pmm.tile([128, 512], F32, name="ph", tag="mm")
                    for k in range(4):
                        kr = X3.shape[0] if k == 3 else 128
                        nc.tensor.matmul(
                            out=ph[:, :cw],
                            lhsT=wh_sb[k][:, m * 128 : (m + 1) * 128],
                            rhs=Xs[k][0:kr, sl],
                            start=(k == 0), stop=(k == 3),
                        )
                    nc.scalar.activation(
                        out=TO[m][:, sl], in_=ph[:, :cw], func=Tanh,
                        bias=bh_sb[:, m : m + 1],
                    )

                # fuse matmul: K = 768 over [TO0..3, CV0, CV1]
                rhs_list = [TO[0], TO[1], TO[2], TO[3], CV0, CV1]
                for m in range(4):
                    pf = pmm.tile([128, 512], F32, name="pf", tag="mm")
                    for k in range(6):
                        nc.tensor.matmul(
                            out=pf[:, :cw],
                            lhsT=wf_sb[k][:, m * 128 : (m + 1) * 128],
                            rhs=rhs_list[k][:, sl],
                            start=(k == 0), stop=(k == 5),
                        )
                    nc.scalar.activation(
                        out=OUT[m][:, sl], in_=pf[:, :cw], func=Tanh,
                        bias=bf_sb[:, m : m + 1],
                    )

            for m in range(4):
                nc.sync.dma_start(
                    out=d_out[m * 128 : (m + 1) * 128, :], in_=OUT[m][:]
                )

    nc.finalize()
    return nc


def _pack(inputs):
    """Host-side routing: group tokens by card_type, build per-core inputs."""
    card_id = np.asarray(inputs["card_id"]).reshape(T)
    card_type = np.asarray(inputs["card_type"]).reshape(T)
    card_cost = np.asarray(inputs["card_cost"]).reshape(T, N_COST)
    sp = np.asarray(inputs["card_special_types"], dtype=np.float32).reshape(T, SP_LEN)
    atk = np.asarray(inputs["atk_n"], dtype=np.float32).reshape(T)
    hp = np.asarray(inputs["hp_n"], dtype=np.float32).reshape(T)
    ha = np.asarray(inputs["has_atk"], dtype=np.float32).reshape(T)
    hh = np.asarray(inputs["has_hp"], dtype=np.float32).reshape(T)
    id_table = np.ascontiguousarray(np.asarray(inputs["id_table"], dtype=np.float32))
    type_table = np.asarray(inputs["type_table"], dtype=np.float32)
    cost_table = np.asarray(inputs["cost_table"], dtype=np.float32)
    Wsp = np.asarray(inputs["Wsp"], dtype=np.float32)
    bsp = np.asarray(inputs["bsp"], dtype=np.float32)
    Wc1 = np.asarray(inputs["Wc1"], dtype=np.float32)
    bc1 = np.asarray(inputs["bc1"], dtype=np.float32)
    Wc2 = np.asarray(inputs["Wc2"], dtype=np.float32)
    bc2 = np.asarray(inputs["bc2"], dtype=np.float32)
    Wh = np.asarray(inputs["Wh"], dtype=np.float32)
    bh = np.asarray(inputs["bh"], dtype=np.float32)
    Wf = np.asarray(inputs["Wf"], dtype=np.float32)
    bf = np.asarray(inputs["bf"], dtype=np.float32)

    toks = [np.nonzero(card_type == t)[0] for t in range(N_CORES)]
    C = max(128, -(-max(len(tk) for tk in toks) // 128) * 128)
    nG = C // 128

    # cont-layer folds: cont_in = [mana, atk, hp, ha, hh, atk+hp] affine in
    # raw rows [cost6, atk, hp, ha, hh, ones] + nonlinear [r1, r2, r3]
    Ma = np.zeros((9, 11), dtype=np.float32)
    Ma[0, 0:6] = 1.0          # mana = sum(cost)
    Ma[1, 6] = 1.0            # atk
    Ma[2, 7] = 1.0            # hp
    Ma[3, 8] = 1.0            # ha
    Ma[4, 9] = 1.0            # hh
    Ma[5, 6] = 1.0            # comb1 = atk + hp
    Ma[5, 7] = 1.0
    wc1a = np.ascontiguousarray(Ma.T @ Wc1)          # [11, 64]
    wc1r1 = np.ascontiguousarray(Wc1[6:7, :])        # comb2 = r1
    wc1r2 = np.ascontiguousarray(Wc1[7:8, :] + Wc1[8:9, :])  # r2*(W7+W8)
    wc1r3 = np.ascontiguousarray(Wc1[8:9, :])        # comb4's r3 part

    in_maps = []
    for t in range(N_CORES):
        tk = toks[t]
        n_t = len(tk)
        ids_pad = np.zeros(C, dtype=np.int32)
        ids_pad[:n_t] = card_id[tk]
        ids_pack = np.ascontiguousarray(ids_pad.reshape(nG, 128).T)  # [128, nG]

        ftsp = np.zeros((32, C), dtype=np.float32)
        ftsp[:, :n_t] = sp[tk].T
        ftc = np.zeros((11, C), dtype=np.float32)
        ftc[0:6, :n_t] = card_cost[tk].T.astype(np.float32)
        ftc[6, :n_t] = atk[tk]
        ftc[7, :n_t] = hp[tk]
        ftc[8, :n_t] = ha[tk]
        ftc[9, :n_t] = hh[tk]
        ftc[10, :] = 1.0  # ones row (eps in mana matmul)

        oh = np.zeros((96, C), dtype=np.float32)
        cc = card_cost[tk]  # [n_t, 6]
        for j in range(N_COST):
            oh[j * COST_LEN + cc[:, j], np.arange(n_t)] = 1.0

        atk2 = np.zeros((1, C), dtype=np.float32)
        atk2[0, :n_t] = atk[tk]
        hp2 = np.zeros((1, C), dtype=np.float32)
        hp2[0, :n_t] = hp[tk]

        # head weight folding
        Wht = Wh[t]  # [640, 512]
        wh_oh = np.concatenate(
            [cost_table @ Wht[320 + 32 * j : 320 + 32 * (j + 1), :] for j in range(N_COST)],
            axis=0,
        )  # [96, 512]
        wh_sp = Wht[512:640, :]
        wh = np.ascontiguousarray(
            np.concatenate([Wht[0:256, :], wh_oh, wh_sp[96:128, :], wh_sp[0:96, :]], axis=0)
        )  # [480, 512]
        bias_head = bh[t] + type_table[t] @ Wht[256:320, :]  # [512]
        bh4 = np.ascontiguousarray(bias_head.reshape(4, 128).T)

        in_maps.append(
            {
                "id_table": id_table,
                "ids": ids_pack,
                "ftsp": ftsp,
                "ftc": ftc,
                "oh": oh,
                "atk2": atk2,
                "hp2": hp2,
                "wh": wh,
                "bh4": bh4,
                "wsp": np.ascontiguousarray(Wsp),
                "bsp": np.ascontiguousarray(bsp.reshape(128, 1)),
                "wc1a": wc1a,
                "wc1r1": wc1r1,
                "wc1r2": wc1r2,
                "wc1r3": wc1r3,
                "bc1": np.ascontiguousarray(bc1.reshape(CONT_HID, 1)),
                "wc2": np.ascontiguousarray(Wc2),
                "bc2p": np.ascontiguousarray(bc2.reshape(2, 128).T),
                "wf": np.ascontiguousarray(Wf),
                "bfp": np.ascontiguousarray(bf.reshape(4, 128).T),
            }
        )
    return C, toks, in_maps


def _unpack(toks, outs):
    full = np.empty((T, EMBED_DIM), dtype=np.float32)
    for t in range(N_CORES):
        n_t = len(toks[t])
        full[toks[t]] = outs[t]["out"][:, :n_t].T
    return full.reshape(B, N, EMBED_DIM)


_PROG_CACHE = {}


def kernel(**inputs) -> np.ndarray:
    global LAST_RESULTS
    C, toks, in_maps = _pack(inputs)

    if os.environ.get("KERNEL_SIM"):
        from concourse.bass_interp import CoreSim

        outs = []
        for t in range(N_CORES):
            nc = _build(C)
            sim = CoreSim(nc)
            for name, val in in_maps[t].items():
                sim.tensor(name)[:] = val
            sim.simulate()
            outs.append({"out": np.array(sim.tensor("out"))})
        return _unpack(toks, outs)

    from concourse.bass_utils import run_bass_kernel_spmd

    if C not in _PROG_CACHE:
        _PROG_CACHE[C] = _build(C)
    nc = _PROG_CACHE[C]
    trace = bool(os.environ.get("BASS_TRACE"))
    res = run_bass_kernel_spmd(
        nc,
        in_maps,
        core_ids=list(range(N_CORES)),
        trace=trace,
        **({"trace_cores": list(range(N_CORES)), "stitch_traces": False} if trace else {}),
    )
    LAST_RESULTS = res
    return _unpack(toks, res.results)


# revision 15
# speedup vs baseline: 2.3196x; 2.3196x over previous
"""CardHandEmbed kernel for 8 Trainium2 NeuronCores.

Strategy (moe_routing): the reference computes all 9 type-heads for every
token and one-hot selects one.  card_type < 8, so only heads 0..7 are ever
selected.  We expert-shard: core t processes exactly the tokens with
card_type == t (routing/permutation done on host), so each core needs only
its own head weights and computes 1/9th of the reference head FLOPs.

Per-core device pipeline (tokens live on the free axis, features on
partitions, i.e. everything is computed transposed):
  - id embedding: indirect-DMA row gather from the (replicated) 100k-row
    table, then PE transposes into feature-major layout.
  - cost embedding: folded into the head matmul.  cost_vec @ Wh_cost ==
    onehot(cost) @ (blockdiag(cost_table)^T @ Wh_cost); the host packs the
    one-hot rows and folds the table into the weight.
  - type embedding: constant per core -> folded into the head bias.
  - cont MLP: the 9 engineered features are affine in
    [cost6, atk, hp, ha, hh] except r1=atk*hp, r2=atk/(mana+eps),
    r3=hp/(mana+eps); the affine part is folded into Wc1 on the host and
    r1..r3 are computed on device and added via rank-1 accumulating matmuls.
  - head matmul (K=480 after folding), tanh, fuse matmul (K=768), tanh.
Matmul operands are bf16 (fp32 matmul runs 2-pass LOW_HIGH on TRN2 = half
rate); PSUM accumulation is fp32 and the final output is fp32.
Host then inverse-permutes the per-core outputs into [B, N, E].
"""

import os

import ml_dtypes
import numpy as np

import concourse.bacc as bacc
import concourse.bass as bass
import concourse.tile as tile
from concourse import mybir

B, N = 64, 128
T = B * N
MAX_ID = 100000
ID_DIM = 256
MAX_TYPE = 8
N_HEADS = MAX_TYPE + 1
TYPE_DIM = 64
COST_LEN = 16
MANA_DIM = 32
N_COST = 6
SP_LEN = 32
CAT_DIM = 128
CONT_HID = 64
EMBED_DIM = 512
HEAD_IN = ID_DIM + TYPE_DIM + MANA_DIM * N_COST + CAT_DIM  # 640
EPS = 1e-6
N_CORES = 8
F32 = mybir.dt.float32
BF16 = mybir.dt.bfloat16
F32R = mybir.dt.float32r
I32 = mybir.dt.int32
NP_BF16 = ml_dtypes.bfloat16

# matmul-operand dtype: f32r = fp32 data on the fast (1 cycle/row, N>=256)
# PE path; bf16 = half storage, ~5e-3 rel err; f32 = exact but 2-pass.
MM_DT_NAME = os.environ.get("KERNEL_MM_DT", "f32r")
MM_DT = {"f32r": F32R, "bf16": BF16, "f32": F32}[MM_DT_NAME]
NP_MM = {"f32r": np.float32, "bf16": NP_BF16, "f32": np.float32}[MM_DT_NAME]

# filled by kernel() for test harness inspection
LAST_RESULTS = None


def _chunks(C, step=512):
    out = []
    c0 = 0
    while c0 < C:
        out.append((c0, min(step, C - c0)))
        c0 += step
    return out


def _build(C):
    """Build the (SPMD-uniform) Bass program for per-core capacity C."""
    nG = C // 128
    nc = bacc.Bacc("TRN2")

    # ---- DRAM I/O ----
    d_table = nc.dram_tensor("id_table", [MAX_ID, ID_DIM], MM_DT, kind="ExternalInput")
    d_ids = nc.dram_tensor("ids", [128, nG], I32, kind="ExternalInput")
    d_ftsp = nc.dram_tensor("ftsp", [32, C], MM_DT, kind="ExternalInput")
    d_ftc = nc.dram_tensor("ftc", [11, C], MM_DT, kind="ExternalInput")
    d_oh = nc.dram_tensor("oh", [96, C], MM_DT, kind="ExternalInput")
    d_atk = nc.dram_tensor("atk2", [1, C], MM_DT, kind="ExternalInput")
    d_hp = nc.dram_tensor("hp2", [1, C], MM_DT, kind="ExternalInput")
    d_wh = nc.dram_tensor("wh", [480, EMBED_DIM], MM_DT, kind="ExternalInput")
    d_bh = nc.dram_tensor("bh4", [128, 4], F32, kind="ExternalInput")
    d_wsp = nc.dram_tensor("wsp", [SP_LEN, CAT_DIM], MM_DT, kind="ExternalInput")
    d_bsp = nc.dram_tensor("bsp", [128, 1], F32, kind="ExternalInput")
    d_wc1a = nc.dram_tensor("wc1a", [11, CONT_HID], MM_DT, kind="ExternalInput")
    d_wc1r1 = nc.dram_tensor("wc1r1", [1, CONT_HID], MM_DT, kind="ExternalInput")
    d_wc1r2 = nc.dram_tensor("wc1r2", [1, CONT_HID], MM_DT, kind="ExternalInput")
    d_wc1r3 = nc.dram_tensor("wc1r3", [1, CONT_HID], MM_DT, kind="ExternalInput")
    d_bc1 = nc.dram_tensor("bc1", [CONT_HID, 1], F32, kind="ExternalInput")
    d_wc2 = nc.dram_tensor("wc2", [CONT_HID, ID_DIM], MM_DT, kind="ExternalInput")
    d_bc2 = nc.dram_tensor("bc2p", [128, 2], F32, kind="ExternalInput")
    d_wf = nc.dram_tensor("wf", [EMBED_DIM + ID_DIM, EMBED_DIM], MM_DT, kind="ExternalInput")
    d_bf = nc.dram_tensor("bfp", [128, 4], F32, kind="ExternalInput")
    d_out = nc.dram_tensor("out", [EMBED_DIM, C], F32, kind="ExternalOutput")

    d_ident = nc.dram_tensor("identc", [128, 128], MM_DT, kind="ExternalInput")
    d_mlhs = nc.dram_tensor("mlhsc", [11, 1], MM_DT, kind="ExternalInput")

    Tanh = mybir.ActivationFunctionType.Tanh

    with tile.TileContext(nc) as tc:
        from contextlib import ExitStack

        with ExitStack() as ctx:
            pers = ctx.enter_context(tc.tile_pool(name="pers", bufs=1))
            gat = ctx.enter_context(tc.tile_pool(name="gat", bufs=4))
            ppt = ctx.enter_context(tc.tile_pool(name="ppt", bufs=2, space="PSUM"))
            pmm = ctx.enter_context(tc.tile_pool(name="pmm", bufs=4, space="PSUM"))
            pmana = ctx.enter_context(tc.tile_pool(name="pmana", bufs=1, space="PSUM"))

            # ---- persistent SBUF tiles + loads ----
            ids_sb = pers.tile([128, nG], I32, name="ids_sb")
            nc.sync.dma_start(out=ids_sb[:], in_=d_ids[:])

            ident_sb = pers.tile([128, 128], MM_DT, name="ident_sb")
            nc.sync.dma_start(out=ident_sb[:], in_=d_ident[:])
            mlhs_sb = pers.tile([11, 1], MM_DT, name="mlhs_sb")
            nc.sync.dma_start(out=mlhs_sb[:], in_=d_mlhs[:])

            ftsp_sb = pers.tile([32, C], MM_DT, name="ftsp_sb")
            nc.sync.dma_start(out=ftsp_sb[:], in_=d_ftsp[:])
            ftc_sb = pers.tile([11, C], MM_DT, name="ftc_sb")
            nc.sync.dma_start(out=ftc_sb[:], in_=d_ftc[:])
            atk_sb = pers.tile([1, C], MM_DT, name="atk_sb")
            nc.sync.dma_start(out=atk_sb[:], in_=d_atk[:])
            hp_sb = pers.tile([1, C], MM_DT, name="hp_sb")
            nc.sync.dma_start(out=hp_sb[:], in_=d_hp[:])

            wsp_sb = pers.tile([SP_LEN, CAT_DIM], MM_DT, name="wsp_sb")
            nc.sync.dma_start(out=wsp_sb[:], in_=d_wsp[:])
            wc1a_sb = pers.tile([11, CONT_HID], MM_DT, name="wc1a_sb")
            nc.sync.dma_start(out=wc1a_sb[:], in_=d_wc1a[:])
            wc1r1_sb = pers.tile([1, CONT_HID], MM_DT, name="wc1r1_sb")
            nc.sync.dma_start(out=wc1r1_sb[:], in_=d_wc1r1[:])
            wc1r2_sb = pers.tile([1, CONT_HID], MM_DT, name="wc1r2_sb")
            nc.sync.dma_start(out=wc1r2_sb[:], in_=d_wc1r2[:])
            wc1r3_sb = pers.tile([1, CONT_HID], MM_DT, name="wc1r3_sb")
            nc.sync.dma_start(out=wc1r3_sb[:], in_=d_wc1r3[:])
            wc2_sb = pers.tile([CONT_HID, ID_DIM], MM_DT, name="wc2_sb")
            nc.sync.dma_start(out=wc2_sb[:], in_=d_wc2[:])
            bsp_sb = pers.tile([128, 1], F32, name="bsp_sb")
            nc.sync.dma_start(out=bsp_sb[:], in_=d_bsp[:])
            bc1_sb = pers.tile([CONT_HID, 1], F32, name="bc1_sb")
            nc.sync.dma_start(out=bc1_sb[:], in_=d_bc1[:])
            bc2_sb = pers.tile([128, 2], F32, name="bc2_sb")
            nc.sync.dma_start(out=bc2_sb[:], in_=d_bc2[:])
            bh_sb = pers.tile([128, 4], F32, name="bh_sb")
            nc.sync.dma_start(out=bh_sb[:], in_=d_bh[:])
            bf_sb = pers.tile([128, 4], F32, name="bf_sb")
            nc.sync.dma_start(out=bf_sb[:], in_=d_bf[:])

            wh_sb = []
            bounds = [(0, 128), (128, 256), (256, 384), (384, 480)]
            for k, (r0, r1) in enumerate(bounds):
                w = pers.tile([r1 - r0, EMBED_DIM], MM_DT, name=f"wh{k}_sb")
                nc.sync.dma_start(out=w[:], in_=d_wh[r0:r1, :])
                wh_sb.append(w)
            wf_sb = []
            for k in range(6):
                w = pers.tile([128, EMBED_DIM], MM_DT, name=f"wf{k}_sb")
                nc.sync.dma_start(out=w[:], in_=d_wf[k * 128 : (k + 1) * 128, :])
                wf_sb.append(w)

            # head-input feature-major tiles (the 4 K-chunks of head matmul)
            X0 = pers.tile([128, C], MM_DT, name="X0")
            X1 = pers.tile([128, C], MM_DT, name="X1")
            X2 = pers.tile([128, C], MM_DT, name="X2")
            X3 = pers.tile([96, C], MM_DT, name="X3")
            nc.sync.dma_start(out=X2[0:96, :], in_=d_oh[:])

            CH = pers.tile([CONT_HID, C], MM_DT, name="CH")
            CV0 = pers.tile([128, C], MM_DT, name="CV0")
            CV1 = pers.tile([128, C], MM_DT, name="CV1")
            RECIP = pers.tile([1, C], MM_DT, name="RECIP")
            R1 = pers.tile([1, C], MM_DT, name="R1")
            R2 = pers.tile([1, C], MM_DT, name="R2")
            R3 = pers.tile([1, C], MM_DT, name="R3")
            TO = [pers.tile([128, C], MM_DT, name=f"TO{m}") for m in range(4)]
            OUT = [pers.tile([128, C], F32, name=f"OUT{m}") for m in range(4)]

            # ---- id gathers (async on gpsimd queue; consumed later) ----
            grows = []
            for g in range(nG):
                rows = gat.tile([128, ID_DIM], MM_DT, name="grows", tag="grows")
                nc.gpsimd.indirect_dma_start(
                    out=rows[:],
                    out_offset=None,
                    in_=d_table[:],
                    in_offset=bass.IndirectOffsetOnAxis(ap=ids_sb[:, g : g + 1], axis=0),
                )
                grows.append(rows)

            # ---- small matmul chain first: warms the PE while gathers land ----
            for c0, cw in _chunks(C):
                sl = slice(c0, c0 + cw)
                # special_vec = tanh(Xsp @ Wsp + bsp), feature-major
                ps = pmm.tile([128, 512], F32, name="ps", tag="mm")
                nc.tensor.matmul(
                    out=ps[:, :cw], lhsT=wsp_sb[:], rhs=ftsp_sb[:, sl],
                    start=True, stop=True,
                )
                nc.scalar.activation(
                    out=X2[96:128, sl], in_=ps[96:128, :cw], func=Tanh,
                    bias=bsp_sb[96:128, :],
                )
                nc.scalar.activation(
                    out=X3[0:96, sl], in_=ps[0:96, :cw], func=Tanh,
                    bias=bsp_sb[0:96, :],
                )

                # mana + eps, reciprocal, nonlinear cont features
                pm = pmana.tile([1, 512], F32, name="pm", tag="pm")
                nc.tensor.matmul(
                    out=pm[:, :cw], lhsT=mlhs_sb[:], rhs=ftc_sb[:, sl],
                    start=True, stop=True,
                )
                with nc.allow_low_precision(reason="bf16/f32r matmul feed rows"):
                    nc.vector.reciprocal(out=RECIP[:, sl], in_=pm[:, :cw])
                    nc.vector.tensor_mul(R1[:, sl], atk_sb[:, sl], hp_sb[:, sl])
                    nc.vector.tensor_mul(R2[:, sl], atk_sb[:, sl], RECIP[:, sl])
                    nc.vector.tensor_mul(R3[:, sl], hp_sb[:, sl], RECIP[:, sl])

                # cont layer 1: affine part + three rank-1 nonlinear rows
                pc1 = pmm.tile([128, 512], F32, name="pc1", tag="mm")
                nc.tensor.matmul(
                    out=pc1[0:CONT_HID, :cw], lhsT=wc1a_sb[:], rhs=ftc_sb[:, sl],
                    start=True, stop=False,
                )
                nc.tensor.matmul(
                    out=pc1[0:CONT_HID, :cw], lhsT=wc1r1_sb[:], rhs=R1[:, sl],
                    start=False, stop=False,
                )
                nc.tensor.matmul(
                    out=pc1[0:CONT_HID, :cw], lhsT=wc1r2_sb[:], rhs=R2[:, sl],
                    start=False, stop=False,
                )
                nc.tensor.matmul(
                    out=pc1[0:CONT_HID, :cw], lhsT=wc1r3_sb[:], rhs=R3[:, sl],
                    start=False, stop=True,
                )
                nc.scalar.activation(
                    out=CH[:, sl], in_=pc1[0:CONT_HID, :cw], func=Tanh, bias=bc1_sb[:],
                )

                # cont layer 2 -> cont_vec (256 dims = CV0, CV1)
                for m, CVm in enumerate((CV0, CV1)):
                    pc2 = pmm.tile([128, 512], F32, name="pc2", tag="mm")
                    nc.tensor.matmul(
                        out=pc2[:, :cw], lhsT=wc2_sb[:, m * 128 : (m + 1) * 128],
                        rhs=CH[:, sl], start=True, stop=True,
                    )
                    nc.scalar.activation(
                        out=CVm[:, sl], in_=pc2[:, :cw], func=Tanh,
                        bias=bc2_sb[:, m : m + 1],
                    )

            # ---- id transposes into X0/X1 (PE, bf16 single-pass) ----
            for g in range(nG):
                for h, Xh in enumerate((X0, X1)):
                    pt = ppt.tile([128, 128], MM_DT, name="pt", tag="pt")
                    nc.tensor.transpose(
                        out=pt[:], in_=grows[g][:, h * 128 : (h + 1) * 128],
                        identity=ident_sb[:],
                    )
                    nc.vector.tensor_copy(
                        out=Xh[:, g * 128 : (g + 1) * 128], in_=pt[:]
                    )

            # ---- head + fuse per chunk ----
            for c0, cw in _chunks(C):
                sl = slice(c0, c0 + cw)
                # routed head matmul: K = 480 over 4 chunks
                Xs = (X0, X1, X2, X3)
                for m in range(4):
                    ph = pmm.tile([128, 512], F32, name="ph", tag="mm")
                    for k in range(4):
                        kr = X3.shape[0] if k == 3 else 128
                        nc.tensor.matmul(
                            out=ph[:, :cw],
                            lhsT=wh_sb[k][:, m * 128 : (m + 1) * 128],
                            rhs=Xs[k][0:kr, sl],
                            start=(k == 0), stop=(k == 3),
                        )
                    nc.scalar.activation(
                        out=TO[m][:, sl], in_=ph[:, :cw], func=Tanh,
                        bias=bh_sb[:, m : m + 1],
                    )

                # fuse matmul: K = 768 over [TO0..3, CV0, CV1]
                rhs_list = [TO[0], TO[1], TO[2], TO[3], CV0, CV1]
                for m in range(4):
                    pf = pmm.tile([128, 512], F32, name="pf", tag="mm")
                    for k in range(6):
                        nc.tensor.matmul(
                            out=pf[:, :cw],
                            lhsT=wf_sb[k][:, m * 128 : (m + 1) * 128],
                            rhs=rhs_list[k][:, sl],
                            start=(k == 0), stop=(k == 5),
                        )
                    nc.scalar.activation(
                        out=OUT[m][:, sl], in_=pf[:, :cw], func=Tanh,
                        bias=bf_sb[:, m : m + 1],
                    )
                    nc.sync.dma_start(
                        out=d_out[m * 128 : (m + 1) * 128, sl], in_=OUT[m][:, sl]
                    )

    nc.finalize()
    return nc


def _pack(inputs):
    """Host-side routing: group tokens by card_type, build per-core inputs."""
    card_id = np.asarray(inputs["card_id"]).reshape(T)
    card_type = np.asarray(inputs["card_type"]).reshape(T)
    card_cost = np.asarray(inputs["card_cost"]).reshape(T, N_COST)
    sp = np.asarray(inputs["card_special_types"], dtype=np.float32).reshape(T, SP_LEN)
    atk = np.asarray(inputs["atk_n"], dtype=np.float32).reshape(T)
    hp = np.asarray(inputs["hp_n"], dtype=np.float32).reshape(T)
    ha = np.asarray(inputs["has_atk"], dtype=np.float32).reshape(T)
    hh = np.asarray(inputs["has_hp"], dtype=np.float32).reshape(T)
    id_table = np.ascontiguousarray(
        np.asarray(inputs["id_table"], dtype=np.float32).astype(NP_MM)
    )
    type_table = np.asarray(inputs["type_table"], dtype=np.float32)
    cost_table = np.asarray(inputs["cost_table"], dtype=np.float32)
    Wsp = np.asarray(inputs["Wsp"], dtype=np.float32)
    bsp = np.asarray(inputs["bsp"], dtype=np.float32)
    Wc1 = np.asarray(inputs["Wc1"], dtype=np.float32)
    bc1 = np.asarray(inputs["bc1"], dtype=np.float32)
    Wc2 = np.asarray(inputs["Wc2"], dtype=np.float32)
    bc2 = np.asarray(inputs["bc2"], dtype=np.float32)
    Wh = np.asarray(inputs["Wh"], dtype=np.float32)
    bh = np.asarray(inputs["bh"], dtype=np.float32)
    Wf = np.asarray(inputs["Wf"], dtype=np.float32)
    bf = np.asarray(inputs["bf"], dtype=np.float32)

    toks = [np.nonzero(card_type == t)[0] for t in range(N_CORES)]
    C = max(128, -(-max(len(tk) for tk in toks) // 128) * 128)
    nG = C // 128

    # cont-layer folds: cont_in = [mana, atk, hp, ha, hh, atk+hp] affine in
    # raw rows [cost6, atk, hp, ha, hh, ones] + nonlinear [r1, r2, r3]
    Ma = np.zeros((9, 11), dtype=np.float32)
    Ma[0, 0:6] = 1.0          # mana = sum(cost)
    Ma[1, 6] = 1.0            # atk
    Ma[2, 7] = 1.0            # hp
    Ma[3, 8] = 1.0            # ha
    Ma[4, 9] = 1.0            # hh
    Ma[5, 6] = 1.0            # comb1 = atk + hp
    Ma[5, 7] = 1.0
    wc1a = np.ascontiguousarray((Ma.T @ Wc1).astype(NP_MM))        # [11, 64]
    wc1r1 = np.ascontiguousarray(Wc1[6:7, :].astype(NP_MM))        # comb2 = r1
    wc1r2 = np.ascontiguousarray((Wc1[7:8, :] + Wc1[8:9, :]).astype(NP_MM))
    wc1r3 = np.ascontiguousarray(Wc1[8:9, :].astype(NP_MM))        # comb4's r3

    in_maps = []
    for t in range(N_CORES):
        tk = toks[t]
        n_t = len(tk)
        ids_pad = np.zeros(C, dtype=np.int32)
        ids_pad[:n_t] = card_id[tk]
        ids_pack = np.ascontiguousarray(ids_pad.reshape(nG, 128).T)  # [128, nG]

        ftsp = np.zeros((32, C), dtype=NP_MM)
        ftsp[:, :n_t] = sp[tk].T.astype(NP_MM)
        ftc = np.zeros((11, C), dtype=NP_MM)
        ftc[0:6, :n_t] = card_cost[tk].T.astype(NP_MM)
        ftc[6, :n_t] = atk[tk].astype(NP_MM)
        ftc[7, :n_t] = hp[tk].astype(NP_MM)
        ftc[8, :n_t] = ha[tk].astype(NP_MM)
        ftc[9, :n_t] = hh[tk].astype(NP_MM)
        ftc[10, :] = 1.0  # ones row (eps in mana matmul)

        oh = np.zeros((96, C), dtype=NP_MM)
        cc = card_cost[tk]  # [n_t, 6]
        for j in range(N_COST):
            oh[j * COST_LEN + cc[:, j], np.arange(n_t)] = 1.0

        atk2 = np.zeros((1, C), dtype=NP_MM)
        atk2[0, :n_t] = atk[tk].astype(NP_MM)
        hp2 = np.zeros((1, C), dtype=NP_MM)
        hp2[0, :n_t] = hp[tk].astype(NP_MM)

        # head weight folding
        Wht = Wh[t]  # [640, 512]
        wh_oh = np.concatenate(
            [cost_table @ Wht[320 + 32 * j : 320 + 32 * (j + 1), :] for j in range(N_COST)],
            axis=0,
        )  # [96, 512]
        wh_sp = Wht[512:640, :]
        wh = np.ascontiguousarray(
            np.concatenate(
                [Wht[0:256, :], wh_oh, wh_sp[96:128, :], wh_sp[0:96, :]], axis=0
            ).astype(NP_MM)
        )  # [480, 512]
        bias_head = bh[t] + type_table[t] @ Wht[256:320, :]  # [512]
        bh4 = np.ascontiguousarray(bias_head.reshape(4, 128).T)

        mlhs_np = np.zeros((11, 1), dtype=NP_MM)
        mlhs_np[0:6, 0] = 1.0
        mlhs_np[10, 0] = EPS
        in_maps.append(
            {
                "identc": np.eye(128, dtype=NP_MM),
                "mlhsc": mlhs_np,
                "id_table": id_table,
                "ids": ids_pack,
                "ftsp": ftsp,
                "ftc": ftc,
                "oh": oh,
                "atk2": atk2,
                "hp2": hp2,
                "wh": wh,
                "bh4": bh4,
                "wsp": np.ascontiguousarray(Wsp.astype(NP_MM)),
                "bsp": np.ascontiguousarray(bsp.reshape(128, 1)),
                "wc1a": wc1a,
                "wc1r1": wc1r1,
                "wc1r2": wc1r2,
                "wc1r3": wc1r3,
                "bc1": np.ascontiguousarray(bc1.reshape(CONT_HID, 1)),
                "wc2": np.ascontiguousarray(Wc2.astype(NP_MM)),
                "bc2p": np.ascontiguousarray(bc2.reshape(2, 128).T),
                "wf": np.ascontiguousarray(Wf.astype(NP_MM)),
                "bfp": np.ascontiguousarray(bf.reshape(4, 128).T),
            }
        )
    return C, toks, in_maps


def _unpack(toks, outs):
    full = np.empty((T, EMBED_DIM), dtype=np.float32)
    for t in range(N_CORES):
        n_t = len(toks[t])
        full[toks[t]] = outs[t]["out"][:, :n_t].T
    return full.reshape(B, N, EMBED_DIM)


_PROG_CACHE = {}


def kernel(**inputs) -> np.ndarray:
    global LAST_RESULTS
    C, toks, in_maps = _pack(inputs)

    if os.environ.get("KERNEL_SIM"):
        from concourse.bass_interp import CoreSim

        outs = []
        for t in range(N_CORES):
            nc = _build(C)
            sim = CoreSim(nc)
            for name, val in in_maps[t].items():
                sim.tensor(name)[:] = val
            sim.simulate()
            outs.append({"out": np.array(sim.tensor("out"))})
        return _unpack(toks, outs)

    from concourse.bass_utils import run_bass_kernel_spmd

    key = (C, MM_DT_NAME)
    if key not in _PROG_CACHE:
        _PROG_CACHE[key] = _build(C)
    nc = _PROG_CACHE[key]
    trace = bool(os.environ.get("BASS_TRACE"))
    res = run_bass_kernel_spmd(
        nc,
        in_maps,
        core_ids=list(range(N_CORES)),
        trace=trace,
        **({"trace_cores": list(range(N_CORES)), "stitch_traces": False} if trace else {}),
    )
    LAST_RESULTS = res
    return _unpack(toks, res.results)
